# revision 10
# baseline (speedup 1.0000x reference)
"""Block-diagonal compress kernel: out = blockdiag(A) @ W @ blockdiag(B).

Shapes (full): W [8192, 8192] f32, A_blocks [128, 64, 64], B_blocks [128, 64, 64].
Sharding: row-shard W / A over 8 cores (1024 rows = 16 A-blocks each);
B replicated.  Each core computes outT = (A_bd @ W_shard @ B_bd)^T and the
host transposes each shard back on gather.

The rel-err gate is 2e-2; bf16 rounding of W/A/B/T costs ~2.5e-3 total and
storing outT as fp8 e3m4 (pre-scaled by 1/32 folded into B, dequantized on
the host) another ~1.33e-2, so W/A/B/T ship in bf16 and the result in fp8:
16 MB W in + 8 MB outT back per core = 24 MB of HBM traffic.

Per-core dataflow (all sizes per core):
  step 1:  T = (A_bd @ W)^T computed 128-column-chunk-wise with W as the
           matmul *stationary* operand:  matmul(lhsT=W[128 rows, 128 cols],
           rhs=blockdiag(A_even^T, A_odd^T)) -> psum [128 cols, 128 rows].
           This absorbs the transpose that a chained matmul otherwise needs.
  step 2:  outT[chunk] = matmul(lhsT=blockdiag(B_j0, B_j1), rhs=T chunk).

Loop structure: 2048-column supergroups (g2 in 4).  For each g2, step 1
fills a bf16 T tile [128, 16384] over 8 row slabs, then step 2 drains it
into 2 output stores of 8 column chunks each.  PSUM->SBUF copies alternate
DVE / ACT (gpsimd has no PSUM port).

DMA layout: W is host-retiled to [4, 8, 128, 2048] bf16 so each W load is
one contiguous 512 KB read with 4 KB per-partition descriptors.  outT is
stored pair-interleaved as [32, 128, 2048] bf16 for 4 KB descriptors too.
W loads ride the SP HWDGE queue; outT stores + preloads the ACT queue.
"""

import bass_rust
import numpy as np

import concourse.bass as bass
import concourse.mybir as mybir
from concourse.bass_utils import run_bass_kernel_spmd
from concourse.tile import TileContext

F32 = mybir.dt.float32
BF16 = mybir.dt.bfloat16
FP8 = mybir.dt.float8e3  # e3m4: 4 mantissa bits, max 15.5

OUT_SCALE = 32.0  # folded into bpack on the host; multiplied back after gather

N_CORES = 8
D = 8192
BLK = 64
ROWS_PC = D // N_CORES  # 1024 rows of W / out per core

_HOIST_OPCODES = {"Matmult", "DMACopy", "TensorCopy", "Memset", "Activation", "Drain"}


def _hoist_excess_matmul_waits(nc: bass.Bass, max_waits: int = 1) -> None:
    """walrus's codegen for several instruction structs (fused-LDWEIGHTS
    matmul, DMA_DIRECT2D, ...) has few sync-wait slots ("Too many sync wait
    commands"). Move excess semaphore waits off such instructions into
    standalone EventSemaphore instructions right before them on the same
    engine queue — the sequencer executes those in order, so the instruction
    still starts only after all waits pass."""
    ctr = 0
    for fnc in nc.m.functions:
        for bb in fnc.blocks:
            new = []
            for ins in bb.instructions:
                si = ins.sync_info if ins.opcode in _HOIST_OPCODES else None
                if si is not None and len(si.on_wait) > max_waits:
                    waits = list(si.on_wait)
                    for w in waits[:-max_waits]:
                        evs = mybir.InstEventSemaphore(
                            name=f"mmwaithoist-{ctr}", ins=[], outs=[]
                        )
                        ctr += 1
                        evs.engine = ins.engine
                        evs.sync_info = bass_rust.SyncInfo(on_wait=[w], on_update=[])
                        new.append(evs)
                    ins.sync_info.on_wait = waits[-max_waits:]
                new.append(ins)
            bb.instructions[:] = new
    return


def build_nc(rows_pc: int = ROWS_PC, d: int = D, hoist: bool = True) -> bass.Bass:
    """One-core SPMD program."""
    R = rows_pc // 128  # 8 row slabs per core (= A-block pairs)
    G2 = d // 2048      # 4 column supergroups
    NQ = d // 256       # 32 output chunk-pairs

    nc = bass.Bass()
    wb_ext = nc.declare_dram_parameter("wb", [G2, R, 128, 2048], BF16, isOutput=False)
    ah_ext = nc.declare_dram_parameter("ah", [128, R * 128], BF16, isOutput=False)
    bp_ext = nc.declare_dram_parameter("bpack", [128, d], BF16, isOutput=False)
    ot_ext = nc.declare_dram_parameter("outt", [NQ, 128, 2048], FP8, isOutput=True)

    with TileContext(nc) as tc:
        with (
            tc.tile_pool(name="const", bufs=1) as cpool,
            tc.tile_pool(name="wp", bufs=6) as wpool,
            tc.tile_pool(name="tg", bufs=2) as tpool,
            tc.tile_pool(name="op", bufs=2) as opool,
            # One shared PSUM pool (4 tiles x 2 banks = all 8 banks): step 1
            # and step 2 draw from the same rotation, so each phase gets the
            # full buffering headroom instead of a static 2+2 split.
            tc.tile_pool(name="ps", bufs=4, space="PSUM") as pspool,
        ):
            p1pool = p2pool = pspool
            # ah rides the ACT queue (needed by the very first matmul);
            # bpack is only needed ~25 us in, so it goes on the otherwise
            # idle gpsimd SWDGE queue to keep startup DMA for W tiles.
            ah = cpool.tile([128, R * 128], BF16)
            nc.scalar.dma_start(out=ah[:], in_=ah_ext[:])
            bpack = cpool.tile([128, d], BF16)
            nc.gpsimd.dma_start(out=bpack[:], in_=bp_ext[:])

            cp = 0  # round-robin DVE/ACT for PSUM->SBUF copies
            for g2 in range(G2):
                # T for this supergroup: bf16 [128, h(2) cc(8) r(8) n(128)];
                # tg[c, h, cc, r, n] = AW^T[g2*2048 + h*1024 + cc*128 + c,
                #                           r*128 + n] for the core's rows.
                tg = tpool.tile([128, 2 * 8 * R * 128], BF16)
                tgv = tg[:].rearrange("p (h cc r n) -> p h cc r n", h=2, cc=8, r=R)
                for r in range(R):
                    wt = wpool.tile([128, 2048], BF16)
                    nc.sync.dma_start(out=wt[:], in_=wb_ext[g2, r])
                    for h in range(2):
                        p1 = p1pool.tile([128, 1024], F32, tag="ps")
                        for cc in range(8):
                            cs = slice(cc * 128, (cc + 1) * 128)
                            ws = slice(h * 1024 + cc * 128, h * 1024 + (cc + 1) * 128)
                            rs = slice(r * 128, (r + 1) * 128)
                            nc.tensor.matmul(
                                p1[:, cs], lhsT=wt[:, ws], rhs=ah[:, rs],
                                start=True, stop=True,
                            )
                        src = p1[:].rearrange("p (cc n) -> p cc n", cc=8)
                        if cp % 2 == 0:
                            nc.vector.tensor_copy(tgv[:, h, :, r, :], src)
                        else:
                            nc.scalar.copy(tgv[:, h, :, r, :], src)
                        cp += 1
                for h in range(2):
                    g = 2 * g2 + h
                    ot = opool.tile([128, 8 * rows_pc], FP8)
                    for cc in range(8):
                        j2 = 8 * g + cc
                        p2 = p2pool.tile([128, rows_pc], F32, tag="ps")
                        lb = bpack[:, j2 * 128 : (j2 + 1) * 128]
                        for s in range(2):
                            w0, w1 = s * 512, (s + 1) * 512
                            ts = slice((h * 8 + cc) * rows_pc + w0,
                                       (h * 8 + cc) * rows_pc + w1)
                            nc.tensor.matmul(
                                p2[:, w0:w1], lhsT=lb, rhs=tg[:, ts],
                                start=True, stop=True,
                            )
                        if cp % 2 == 0:
                            nc.vector.tensor_copy(
                                ot[:, cc * rows_pc : (cc + 1) * rows_pc], p2[:]
                            )
                        else:
                            nc.scalar.copy(
                                ot[:, cc * rows_pc : (cc + 1) * rows_pc], p2[:]
                            )
                        cp += 1
                    # 8 chunks j2 in [8g, 8g+8) = DRAM rows q in [4g, 4g+4)
                    nc.scalar.dma_start(
                        out=ot_ext[4 * g : 4 * (g + 1)].transpose([1, 0, 2]),
                        in_=ot[:].rearrange("p (q n) -> p q n", q=4),
                    )
    if hoist:
        _hoist_excess_matmul_waits(nc)
    return nc


def pack_at(a_blocks: np.ndarray) -> np.ndarray:
    """[2R, 64, 64] A blocks -> [128, R*128] with
    out[64*b + k, 128*r + 64*b + n] = A[2r+b][n, k] (transposed, pair-blockdiag)."""
    nb = a_blocks.shape[0]
    R = nb // 2
    out = np.zeros((128, R * 128), np.float32)
    at = a_blocks.transpose(0, 2, 1)
    out[0:64].reshape(64, R, 2, 64)[:, :, 0, :] = at[0::2].transpose(1, 0, 2)
    out[64:128].reshape(64, R, 2, 64)[:, :, 1, :] = at[1::2].transpose(1, 0, 2)
    return out


def pack_b(b_blocks: np.ndarray) -> np.ndarray:
    """[2J, 64, 64] B blocks -> [128, J*128] with
    out[64*b + k, 128*j + 64*b + n] = B[2j+b][k, n] (pair-blockdiag, untransposed)."""
    nb = b_blocks.shape[0]
    J = nb // 2
    out = np.zeros((128, J * 128), np.float32)
    out[0:64].reshape(64, J, 2, 64)[:, :, 0, :] = b_blocks[0::2].transpose(1, 0, 2)
    out[64:128].reshape(64, J, 2, 64)[:, :, 1, :] = b_blocks[1::2].transpose(1, 0, 2)
    return out


def pack_w(w_shard: np.ndarray):
    """[rows_pc, d] f32 -> bf16 [G2, R, 128, 2048] so each (g2, r) W tile is
    one contiguous 512 KB block in DRAM with 4 KB per-partition lines."""
    import ml_dtypes

    rows_pc, d = w_shard.shape
    R, G2 = rows_pc // 128, d // 2048
    return np.ascontiguousarray(
        w_shard.reshape(R, 128, G2, 2048).transpose(2, 0, 1, 3)
    ).astype(ml_dtypes.bfloat16)


def unpack_out(ot: np.ndarray) -> np.ndarray:
    """[NQ, 128, 2048] fp8 pair-interleaved outT/OUT_SCALE -> [rows_pc, d] f32."""
    nq = ot.shape[0]
    outt = (
        (np.asarray(ot, dtype=np.float32) * OUT_SCALE)
        .reshape(nq, 128, 2, 1024)
        .transpose(0, 2, 1, 3)
        .reshape(nq * 256, 1024)
    )
    return outt.T


_NC_CACHE: dict = {}


def run(W, A_blocks, B_blocks, trace: bool = False, trace_cores=None):
    import ml_dtypes

    W = np.asarray(W, dtype=np.float32)
    A_blocks = np.asarray(A_blocks, dtype=np.float32)
    B_blocks = np.asarray(B_blocks, dtype=np.float32)
    assert W.shape == (D, D) and A_blocks.shape == (D // BLK, BLK, BLK)

    if "nc" not in _NC_CACHE:
        _NC_CACHE["nc"] = build_nc()
    nc = _NC_CACHE["nc"]

    bp = (pack_b(B_blocks) / OUT_SCALE).astype(ml_dtypes.bfloat16)
    in_maps = []
    for c in range(N_CORES):
        wb = pack_w(W[ROWS_PC * c : ROWS_PC * (c + 1)])
        ah = pack_at(A_blocks[16 * c : 16 * (c + 1)]).astype(ml_dtypes.bfloat16)
        in_maps.append({"wb": wb, "ah": ah, "bpack": bp})
    res = run_bass_kernel_spmd(
        nc, in_maps, core_ids=list(range(N_CORES)), trace=trace, trace_cores=trace_cores
    )
    out = np.empty((D, D), np.float32)
    for c in range(N_CORES):
        out[ROWS_PC * c : ROWS_PC * (c + 1), :] = unpack_out(res.results[c]["outt"])
    return out, res


def kernel(W, A_blocks, B_blocks):
    out, _ = run(W, A_blocks, B_blocks, trace=False)
    return out


# revision 14
# speedup vs baseline: 1.2180x; 1.2180x over previous
"""Block-diagonal compress kernel: out = blockdiag(A) @ W @ blockdiag(B).

Shapes (full): W [8192, 8192] f32, A_blocks [128, 64, 64], B_blocks [128, 64, 64].
Sharding: row-shard W / A over 8 cores (1024 rows = 16 A-blocks each);
B replicated.  Each core computes outT = (A_bd @ W_shard @ B_bd)^T and the
host transposes each shard back on gather.

The rel-err gate is 2e-2; bf16 rounding of W/A/B/T costs ~2.5e-3 total and
storing outT as fp8 e3m4 (pre-scaled by 1/32 folded into B, dequantized on
the host) another ~1.33e-2, so W/A/B/T ship in bf16 and the result in fp8:
16 MB W in + 8 MB outT back per core = 24 MB of HBM traffic.

Per-core dataflow (all sizes per core):
  step 1:  T = (A_bd @ W)^T computed 128-column-chunk-wise with W as the
           matmul *stationary* operand:  matmul(lhsT=W[128 rows, 128 cols],
           rhs=blockdiag(A_even^T, A_odd^T)) -> psum [128 cols, 128 rows].
           This absorbs the transpose that a chained matmul otherwise needs.
  step 2:  outT[chunk] = matmul(lhsT=blockdiag(B_j0, B_j1), rhs=T chunk).

Loop structure: 2048-column supergroups (g2 in 4).  For each g2, step 1
fills a bf16 T tile [128, 16384] over 8 row slabs, then step 2 drains it
into 2 output stores of 8 column chunks each.  PSUM->SBUF copies alternate
DVE / ACT (gpsimd has no PSUM port).

DMA layout: W is host-retiled to [4, 8, 128, 2048] bf16 so each W load is
one contiguous 512 KB read with 4 KB per-partition descriptors.  outT is
stored pair-interleaved as [32, 128, 2048] bf16 for 4 KB descriptors too.
W loads ride the SP HWDGE queue; outT stores + preloads the ACT queue.
"""

import bass_rust
import numpy as np

import concourse.bass as bass
import concourse.mybir as mybir
from concourse.bass_utils import run_bass_kernel_spmd
from concourse.tile import TileContext

F32 = mybir.dt.float32
BF16 = mybir.dt.bfloat16
FP8 = mybir.dt.float8e3  # e3m4: 4 mantissa bits, max 15.5

OUT_SCALE = 32.0  # folded into bpack on the host; multiplied back after gather

N_CORES = 8
D = 8192
BLK = 64
ROWS_PC = D // N_CORES  # 1024 rows of W / out per core

_HOIST_OPCODES = {"Matmult", "DMACopy", "TensorCopy", "Memset", "Activation", "Drain"}


def _hoist_excess_matmul_waits(nc: bass.Bass, max_waits: int = 1) -> None:
    """walrus's codegen for several instruction structs (fused-LDWEIGHTS
    matmul, DMA_DIRECT2D, ...) has few sync-wait slots ("Too many sync wait
    commands"). Move excess semaphore waits off such instructions into
    standalone EventSemaphore instructions right before them on the same
    engine queue — the sequencer executes those in order, so the instruction
    still starts only after all waits pass."""
    ctr = 0
    for fnc in nc.m.functions:
        for bb in fnc.blocks:
            new = []
            for ins in bb.instructions:
                si = ins.sync_info if ins.opcode in _HOIST_OPCODES else None
                if si is not None and len(si.on_wait) > max_waits:
                    waits = list(si.on_wait)
                    for w in waits[:-max_waits]:
                        evs = mybir.InstEventSemaphore(
                            name=f"mmwaithoist-{ctr}", ins=[], outs=[]
                        )
                        ctr += 1
                        evs.engine = ins.engine
                        evs.sync_info = bass_rust.SyncInfo(on_wait=[w], on_update=[])
                        new.append(evs)
                    ins.sync_info.on_wait = waits[-max_waits:]
                new.append(ins)
            bb.instructions[:] = new
    return


def build_nc(rows_pc: int = ROWS_PC, d: int = D, hoist: bool = True) -> bass.Bass:
    """One-core SPMD program."""
    R = rows_pc // 128  # 8 row slabs per core (= A-block pairs)
    G2 = d // 2048      # 4 column supergroups
    NQ = d // 256       # 32 output chunk-pairs

    nc = bass.Bass()
    wb_ext = nc.declare_dram_parameter("wb", [G2, R, 128, 2048], BF16, isOutput=False)
    ah_ext = nc.declare_dram_parameter("ah", [128, R * 128], BF16, isOutput=False)
    bp_ext = nc.declare_dram_parameter("bpack", [128, d], BF16, isOutput=False)
    ot_ext = nc.declare_dram_parameter("outt", [NQ, 128, 2048], FP8, isOutput=True)

    with TileContext(nc) as tc:
        with (
            tc.tile_pool(name="const", bufs=1) as cpool,
            tc.tile_pool(name="wp", bufs=6) as wpool,
            tc.tile_pool(name="tg", bufs=2) as tpool,
            tc.tile_pool(name="op", bufs=2) as opool,
            # One shared PSUM pool (4 tiles x 2 banks = all 8 banks): step 1
            # and step 2 draw from the same rotation, so each phase gets the
            # full buffering headroom instead of a static 2+2 split.
            tc.tile_pool(name="ps", bufs=4, space="PSUM") as pspool,
        ):
            p1pool = p2pool = pspool
            # ah rides the ACT queue (needed by the very first matmul).
            # bpack is loaded in 2048-column quarters: quarter 0 up front,
            # quarter g2+1 after phase g2's stores — each arrives one phase
            # before step 2 needs it without crowding the startup DMA window
            # (a single 2 MB preload starves the first W tiles for ~9 us).
            ah = cpool.tile([128, R * 128], BF16)
            nc.scalar.dma_start(out=ah[:], in_=ah_ext[:])
            bpack = cpool.tile([128, d], BF16)
            nc.scalar.dma_start(out=bpack[:, 0:2048], in_=bp_ext[:, 0:2048])

            # Greedy DVE/ACT balance with measured per-copy costs (ns):
            # psum->bf16 1199/1100, psum->fp8 1600/1520.
            busy = [0.0, 0.0]  # [DVE, ACT]

            def copy(dst, src, cost):
                e = 0 if busy[0] + cost[0] <= busy[1] + cost[1] else 1
                if e == 0:
                    nc.vector.tensor_copy(dst, src)
                else:
                    nc.scalar.copy(dst, src)
                busy[e] += cost[e]

            for g2 in range(G2):
                # T for this supergroup: bf16 [128, h(2) cc(8) r(8) n(128)];
                # tg[c, h, cc, r, n] = AW^T[g2*2048 + h*1024 + cc*128 + c,
                #                           r*128 + n] for the core's rows.
                tg = tpool.tile([128, 2 * 8 * R * 128], BF16)
                tgv = tg[:].rearrange("p (h cc r n) -> p h cc r n", h=2, cc=8, r=R)
                for r in range(R):
                    wt = wpool.tile([128, 2048], BF16)
                    nc.sync.dma_start(out=wt[:], in_=wb_ext[g2, r])
                    for h in range(2):
                        p1 = p1pool.tile([128, 1024], F32, tag="ps")
                        for cc in range(8):
                            cs = slice(cc * 128, (cc + 1) * 128)
                            ws = slice(h * 1024 + cc * 128, h * 1024 + (cc + 1) * 128)
                            rs = slice(r * 128, (r + 1) * 128)
                            nc.tensor.matmul(
                                p1[:, cs], lhsT=wt[:, ws], rhs=ah[:, rs],
                                start=True, stop=True,
                            )
                        src = p1[:].rearrange("p (cc n) -> p cc n", cc=8)
                        if r == R - 1:
                            # these copies gate step 2 of this supergroup:
                            # split across both engines to halve the latency
                            nc.vector.tensor_copy(
                                tgv[:, h, 0:4, r, :], src[:, 0:4, :]
                            )
                            nc.scalar.copy(tgv[:, h, 4:8, r, :], src[:, 4:8, :])
                            busy[0] += 663.0
                            busy[1] += 709.0
                        else:
                            copy(tgv[:, h, :, r, :], src, (1199.0, 1100.0))
                for h in range(2):
                    g = 2 * g2 + h
                    ot = opool.tile([128, 8 * rows_pc], FP8)
                    last = g2 == G2 - 1 and h == 1
                    for cc in range(8):
                        j2 = 8 * g + cc
                        p2 = p2pool.tile([128, rows_pc], F32, tag="ps")
                        lb = bpack[:, j2 * 128 : (j2 + 1) * 128]
                        for s in range(2):
                            w0, w1 = s * 512, (s + 1) * 512
                            ts = slice((h * 8 + cc) * rows_pc + w0,
                                       (h * 8 + cc) * rows_pc + w1)
                            nc.tensor.matmul(
                                p2[:, w0:w1], lhsT=lb, rhs=tg[:, ts],
                                start=True, stop=True,
                            )
                        copy(
                            ot[:, cc * rows_pc : (cc + 1) * rows_pc], p2[:],
                            (1600.0, 1520.0),
                        )
                        if last and cc == 3:
                            # drain the final phase in two half stores so the
                            # tail overlaps the remaining copies
                            nc.scalar.dma_start(
                                out=ot_ext[4 * g : 4 * g + 2].transpose([1, 0, 2]),
                                in_=ot[:, 0 : 4 * rows_pc].rearrange(
                                    "p (q n) -> p q n", q=2
                                ),
                            )
                    # 8 chunks j2 in [8g, 8g+8) = DRAM rows q in [4g, 4g+4)
                    if last:
                        nc.scalar.dma_start(
                            out=ot_ext[4 * g + 2 : 4 * g + 4].transpose([1, 0, 2]),
                            in_=ot[:, 4 * rows_pc :].rearrange(
                                "p (q n) -> p q n", q=2
                            ),
                        )
                    else:
                        nc.scalar.dma_start(
                            out=ot_ext[4 * g : 4 * (g + 1)].transpose([1, 0, 2]),
                            in_=ot[:].rearrange("p (q n) -> p q n", q=4),
                        )
                if g2 < G2 - 1:
                    cs2 = slice((g2 + 1) * 2048, (g2 + 2) * 2048)
                    nc.scalar.dma_start(out=bpack[:, cs2], in_=bp_ext[:, cs2])
    if hoist:
        _hoist_excess_matmul_waits(nc)
    return nc


def pack_at(a_blocks: np.ndarray) -> np.ndarray:
    """[2R, 64, 64] A blocks -> [128, R*128] with
    out[64*b + k, 128*r + 64*b + n] = A[2r+b][n, k] (transposed, pair-blockdiag)."""
    nb = a_blocks.shape[0]
    R = nb // 2
    out = np.zeros((128, R * 128), np.float32)
    at = a_blocks.transpose(0, 2, 1)
    out[0:64].reshape(64, R, 2, 64)[:, :, 0, :] = at[0::2].transpose(1, 0, 2)
    out[64:128].reshape(64, R, 2, 64)[:, :, 1, :] = at[1::2].transpose(1, 0, 2)
    return out


def pack_b(b_blocks: np.ndarray) -> np.ndarray:
    """[2J, 64, 64] B blocks -> [128, J*128] with
    out[64*b + k, 128*j + 64*b + n] = B[2j+b][k, n] (pair-blockdiag, untransposed)."""
    nb = b_blocks.shape[0]
    J = nb // 2
    out = np.zeros((128, J * 128), np.float32)
    out[0:64].reshape(64, J, 2, 64)[:, :, 0, :] = b_blocks[0::2].transpose(1, 0, 2)
    out[64:128].reshape(64, J, 2, 64)[:, :, 1, :] = b_blocks[1::2].transpose(1, 0, 2)
    return out


def pack_w(w_shard: np.ndarray):
    """[rows_pc, d] f32 -> bf16 [G2, R, 128, 2048] so each (g2, r) W tile is
    one contiguous 512 KB block in DRAM with 4 KB per-partition lines."""
    import ml_dtypes

    rows_pc, d = w_shard.shape
    R, G2 = rows_pc // 128, d // 2048
    return np.ascontiguousarray(
        w_shard.reshape(R, 128, G2, 2048).transpose(2, 0, 1, 3)
    ).astype(ml_dtypes.bfloat16)


def unpack_out(ot: np.ndarray) -> np.ndarray:
    """[NQ, 128, 2048] fp8 pair-interleaved outT/OUT_SCALE -> [rows_pc, d] f32."""
    nq = ot.shape[0]
    outt = (
        (np.asarray(ot, dtype=np.float32) * OUT_SCALE)
        .reshape(nq, 128, 2, 1024)
        .transpose(0, 2, 1, 3)
        .reshape(nq * 256, 1024)
    )
    return outt.T


_NC_CACHE: dict = {}


def run(W, A_blocks, B_blocks, trace: bool = False, trace_cores=None):
    import ml_dtypes

    W = np.asarray(W, dtype=np.float32)
    A_blocks = np.asarray(A_blocks, dtype=np.float32)
    B_blocks = np.asarray(B_blocks, dtype=np.float32)
    assert W.shape == (D, D) and A_blocks.shape == (D // BLK, BLK, BLK)

    if "nc" not in _NC_CACHE:
        _NC_CACHE["nc"] = build_nc()
    nc = _NC_CACHE["nc"]

    bp = (pack_b(B_blocks) / OUT_SCALE).astype(ml_dtypes.bfloat16)
    in_maps = []
    for c in range(N_CORES):
        wb = pack_w(W[ROWS_PC * c : ROWS_PC * (c + 1)])
        ah = pack_at(A_blocks[16 * c : 16 * (c + 1)]).astype(ml_dtypes.bfloat16)
        in_maps.append({"wb": wb, "ah": ah, "bpack": bp})
    res = run_bass_kernel_spmd(
        nc, in_maps, core_ids=list(range(N_CORES)), trace=trace, trace_cores=trace_cores
    )
    out = np.empty((D, D), np.float32)
    for c in range(N_CORES):
        out[ROWS_PC * c : ROWS_PC * (c + 1), :] = unpack_out(res.results[c]["outt"])
    return out, res


def kernel(W, A_blocks, B_blocks):
    out, _ = run(W, A_blocks, B_blocks, trace=False)
    return out


# revision 21
# speedup vs baseline: 1.2858x; 1.0556x over previous
"""Block-diagonal compress kernel: out = blockdiag(A) @ W @ blockdiag(B).

Shapes (full): W [8192, 8192] f32, A_blocks [128, 64, 64], B_blocks [128, 64, 64].
Sharding: row-shard W / A over 8 cores (1024 rows = 16 A-blocks each);
B replicated.  Each core computes outT = (A_bd @ W_shard @ B_bd)^T and the
host transposes each shard back on gather.

The rel-err gate is 2e-2; bf16 rounding of W/A/B/T costs ~2.5e-3 total and
storing outT as fp8 e3m4 (pre-scaled by 1/32 folded into B, dequantized on
the host) another ~1.33e-2, so W/A/B/T ship in bf16 and the result in fp8:
16 MB W in + 8 MB outT back per core = 24 MB of HBM traffic.

Per-core dataflow (all sizes per core):
  step 1:  T = (A_bd @ W)^T computed 128-column-chunk-wise with W as the
           matmul *stationary* operand:  matmul(lhsT=W[128 rows, 128 cols],
           rhs=blockdiag(A_even^T, A_odd^T)) -> psum [128 cols, 128 rows].
           This absorbs the transpose that a chained matmul otherwise needs.
  step 2:  outT[chunk] = matmul(lhsT=blockdiag(B_j0, B_j1), rhs=T chunk).

Loop structure: 2048-column supergroups (g2 in 4).  For each g2, step 1
fills a bf16 T tile [128, 16384] over 8 row slabs, then step 2 drains it
into 2 output stores of 8 column chunks each.  PSUM->SBUF copies alternate
DVE / ACT (gpsimd has no PSUM port).

DMA layout: W is host-retiled to [4, 8, 128, 2048] bf16 so each W load is
one contiguous 512 KB read with 4 KB per-partition descriptors.  outT is
stored pair-interleaved as [32, 128, 2048] bf16 for 4 KB descriptors too.
W loads ride the SP HWDGE queue; outT stores + preloads the ACT queue.
"""

import bass_rust
import numpy as np

import concourse.bass as bass
import concourse.mybir as mybir
from concourse.bass_utils import run_bass_kernel_spmd
from concourse.tile import TileContext

F32 = mybir.dt.float32
BF16 = mybir.dt.bfloat16
FP8 = mybir.dt.float8e3  # e3m4: 4 mantissa bits, max 15.5

OUT_SCALE = 32.0  # folded into bpack on the host; multiplied back after gather

N_CORES = 8
D = 8192
BLK = 64
ROWS_PC = D // N_CORES  # 1024 rows of W / out per core

_HOIST_OPCODES = {"Matmult", "DMACopy", "TensorCopy", "Memset", "Activation", "Drain"}


def _hoist_excess_matmul_waits(nc: bass.Bass, max_waits: int = 1) -> None:
    """walrus's codegen for several instruction structs (fused-LDWEIGHTS
    matmul, DMA_DIRECT2D, ...) has few sync-wait slots ("Too many sync wait
    commands"). Move excess semaphore waits off such instructions into
    standalone EventSemaphore instructions right before them on the same
    engine queue — the sequencer executes those in order, so the instruction
    still starts only after all waits pass."""
    ctr = 0
    for fnc in nc.m.functions:
        for bb in fnc.blocks:
            new = []
            for ins in bb.instructions:
                si = ins.sync_info if ins.opcode in _HOIST_OPCODES else None
                if si is not None and len(si.on_wait) > max_waits:
                    waits = list(si.on_wait)
                    for w in waits[:-max_waits]:
                        evs = mybir.InstEventSemaphore(
                            name=f"mmwaithoist-{ctr}", ins=[], outs=[]
                        )
                        ctr += 1
                        evs.engine = ins.engine
                        evs.sync_info = bass_rust.SyncInfo(on_wait=[w], on_update=[])
                        new.append(evs)
                    ins.sync_info.on_wait = waits[-max_waits:]
                new.append(ins)
            bb.instructions[:] = new
    return


def build_nc(rows_pc: int = ROWS_PC, d: int = D, hoist: bool = True) -> bass.Bass:
    """One-core SPMD program."""
    R = rows_pc // 128  # 8 row slabs per core (= A-block pairs)
    G2 = d // 2048      # 4 column supergroups
    NQ = d // 256       # 32 output chunk-pairs

    nc = bass.Bass()
    wb_ext = nc.declare_dram_parameter("wb", [G2, R, 128, 2048], BF16, isOutput=False)
    ah_ext = nc.declare_dram_parameter("ah", [128, R * 128], BF16, isOutput=False)
    bp_ext = nc.declare_dram_parameter("bpack", [128, d], BF16, isOutput=False)
    ot_ext = nc.declare_dram_parameter("outt", [NQ, 128, 2048], FP8, isOutput=True)

    with TileContext(nc) as tc:
        with (
            tc.tile_pool(name="const", bufs=1) as cpool,
            tc.tile_pool(name="wp", bufs=8) as wpool,
            tc.tile_pool(name="tg", bufs=2) as tpool,
            tc.tile_pool(name="op", bufs=2) as opool,
            # One shared PSUM pool (4 tiles x 2 banks = all 8 banks): step 1
            # and step 2 draw from the same rotation, so each phase gets the
            # full buffering headroom instead of a static 2+2 split.
            tc.tile_pool(name="ps", bufs=4, space="PSUM") as pspool,
        ):
            p1pool = p2pool = pspool
            # ah rides the ACT queue (needed by the very first matmul).
            # bpack is loaded in 2048-column quarters: quarter 0 up front,
            # quarter g2+1 after phase g2's stores — each arrives one phase
            # before step 2 needs it without crowding the startup DMA window
            # (a single 2 MB preload starves the first W tiles for ~9 us).
            # ah on the gpsimd SWDGE queue and the first W tile on the ACT
            # queue so both land while the SP queue spins up its stream.
            ah = cpool.tile([128, R * 128], BF16)
            nc.gpsimd.dma_start(out=ah[:], in_=ah_ext[:])
            bpack = cpool.tile([128, d], BF16)

            # Greedy DVE/ACT balance with measured per-copy costs (ns):
            # psum->bf16 1199/1100, psum->fp8 1600/1520.
            busy = [0.0, 0.0]  # [DVE, ACT]

            def copy(dst, src, cost):
                e = 0 if busy[0] + cost[0] <= busy[1] + cost[1] else 1
                if e == 0:
                    nc.vector.tensor_copy(dst, src)
                else:
                    nc.scalar.copy(dst, src)
                busy[e] += cost[e]

            for g2 in range(G2):
                # T for this supergroup: bf16 [128, h(2) cc(8) r(8) n(128)];
                # tg[c, h, cc, r, n] = AW^T[g2*2048 + h*1024 + cc*128 + c,
                #                           r*128 + n] for the core's rows.
                tg = tpool.tile([128, 2 * 8 * R * 128], BF16)
                tgv = tg[:].rearrange("p (h cc r n) -> p h cc r n", h=2, cc=8, r=R)
                for r in range(R):
                    wt = wpool.tile([128, 2048], BF16)
                    if g2 == 0 and r == 0:
                        # first tile gates the first matmul: fetch it on the
                        # ACT queue in parallel with SP starting tiles 1+,
                        # then queue bpack's first quarter behind it
                        nc.scalar.dma_start(out=wt[:], in_=wb_ext[g2, r])
                        nc.scalar.dma_start(
                            out=bpack[:, 0:2048], in_=bp_ext[:, 0:2048]
                        )
                        busy[1] += 1400.0
                    else:
                        nc.sync.dma_start(out=wt[:], in_=wb_ext[g2, r])
                    for h in range(2):
                        p1 = p1pool.tile([128, 1024], F32, tag="ps")
                        for cc in range(8):
                            cs = slice(cc * 128, (cc + 1) * 128)
                            ws = slice(h * 1024 + cc * 128, h * 1024 + (cc + 1) * 128)
                            rs = slice(r * 128, (r + 1) * 128)
                            nc.tensor.matmul(
                                p1[:, cs], lhsT=wt[:, ws], rhs=ah[:, rs],
                                start=True, stop=True,
                            )
                        src = p1[:].rearrange("p (cc n) -> p cc n", cc=8)
                        if r == R - 1:
                            # these copies gate step 2 of this supergroup:
                            # split across both engines to halve the latency
                            nc.vector.tensor_copy(
                                tgv[:, h, 0:4, r, :], src[:, 0:4, :]
                            )
                            nc.scalar.copy(tgv[:, h, 4:8, r, :], src[:, 4:8, :])
                            busy[0] += 663.0
                            busy[1] += 709.0
                        else:
                            copy(tgv[:, h, :, r, :], src, (1199.0, 1100.0))
                for h in range(2):
                    g = 2 * g2 + h
                    ot = opool.tile([128, 8 * rows_pc], FP8)
                    last = g2 == G2 - 1 and h == 1
                    for cc in range(8):
                        j2 = 8 * g + cc
                        p2 = p2pool.tile([128, rows_pc], F32, tag="ps")
                        lb = bpack[:, j2 * 128 : (j2 + 1) * 128]
                        for s in range(2):
                            w0, w1 = s * 512, (s + 1) * 512
                            ts = slice((h * 8 + cc) * rows_pc + w0,
                                       (h * 8 + cc) * rows_pc + w1)
                            nc.tensor.matmul(
                                p2[:, w0:w1], lhsT=lb, rhs=tg[:, ts],
                                start=True, stop=True,
                            )
                        copy(
                            ot[:, cc * rows_pc : (cc + 1) * rows_pc], p2[:],
                            (1600.0, 1520.0),
                        )
                        if last and cc % 2 == 1:
                            # drain the final phase in four quarter stores so
                            # the tail overlaps the remaining copies
                            q0 = 4 * g + cc // 2
                            nc.scalar.dma_start(
                                out=ot_ext[q0 : q0 + 1].transpose([1, 0, 2]),
                                in_=ot[
                                    :, (cc - 1) * rows_pc : (cc + 1) * rows_pc
                                ].rearrange("p (q n) -> p q n", q=1),
                            )
                            busy[1] += 700.0
                    # 8 chunks j2 in [8g, 8g+8) = DRAM rows q in [4g, 4g+4)
                    if not last:
                        nc.scalar.dma_start(
                            out=ot_ext[4 * g : 4 * (g + 1)].transpose([1, 0, 2]),
                            in_=ot[:].rearrange("p (q n) -> p q n", q=4),
                        )
                        busy[1] += 700.0
                if g2 < G2 - 1:
                    cs2 = slice((g2 + 1) * 2048, (g2 + 2) * 2048)
                    nc.scalar.dma_start(out=bpack[:, cs2], in_=bp_ext[:, cs2])
                    busy[1] += 700.0
    if hoist:
        _hoist_excess_matmul_waits(nc)
    return nc


def pack_at(a_blocks: np.ndarray) -> np.ndarray:
    """[2R, 64, 64] A blocks -> [128, R*128] with
    out[64*b + k, 128*r + 64*b + n] = A[2r+b][n, k] (transposed, pair-blockdiag)."""
    nb = a_blocks.shape[0]
    R = nb // 2
    out = np.zeros((128, R * 128), np.float32)
    at = a_blocks.transpose(0, 2, 1)
    out[0:64].reshape(64, R, 2, 64)[:, :, 0, :] = at[0::2].transpose(1, 0, 2)
    out[64:128].reshape(64, R, 2, 64)[:, :, 1, :] = at[1::2].transpose(1, 0, 2)
    return out


def pack_b(b_blocks: np.ndarray) -> np.ndarray:
    """[2J, 64, 64] B blocks -> [128, J*128] with
    out[64*b + k, 128*j + 64*b + n] = B[2j+b][k, n] (pair-blockdiag, untransposed)."""
    nb = b_blocks.shape[0]
    J = nb // 2
    out = np.zeros((128, J * 128), np.float32)
    out[0:64].reshape(64, J, 2, 64)[:, :, 0, :] = b_blocks[0::2].transpose(1, 0, 2)
    out[64:128].reshape(64, J, 2, 64)[:, :, 1, :] = b_blocks[1::2].transpose(1, 0, 2)
    return out


def pack_w(w_shard: np.ndarray):
    """[rows_pc, d] f32 -> bf16 [G2, R, 128, 2048] so each (g2, r) W tile is
    one contiguous 512 KB block in DRAM with 4 KB per-partition lines."""
    import ml_dtypes

    rows_pc, d = w_shard.shape
    R, G2 = rows_pc // 128, d // 2048
    return np.ascontiguousarray(
        w_shard.reshape(R, 128, G2, 2048).transpose(2, 0, 1, 3)
    ).astype(ml_dtypes.bfloat16)


def unpack_out(ot: np.ndarray) -> np.ndarray:
    """[NQ, 128, 2048] fp8 pair-interleaved outT/OUT_SCALE -> [rows_pc, d] f32."""
    nq = ot.shape[0]
    outt = (
        (np.asarray(ot, dtype=np.float32) * OUT_SCALE)
        .reshape(nq, 128, 2, 1024)
        .transpose(0, 2, 1, 3)
        .reshape(nq * 256, 1024)
    )
    return outt.T


_NC_CACHE: dict = {}


def run(W, A_blocks, B_blocks, trace: bool = False, trace_cores=None):
    import ml_dtypes

    W = np.asarray(W, dtype=np.float32)
    A_blocks = np.asarray(A_blocks, dtype=np.float32)
    B_blocks = np.asarray(B_blocks, dtype=np.float32)
    assert W.shape == (D, D) and A_blocks.shape == (D // BLK, BLK, BLK)

    if "nc" not in _NC_CACHE:
        _NC_CACHE["nc"] = build_nc()
    nc = _NC_CACHE["nc"]

    bp = (pack_b(B_blocks) / OUT_SCALE).astype(ml_dtypes.bfloat16)
    in_maps = []
    for c in range(N_CORES):
        wb = pack_w(W[ROWS_PC * c : ROWS_PC * (c + 1)])
        ah = pack_at(A_blocks[16 * c : 16 * (c + 1)]).astype(ml_dtypes.bfloat16)
        in_maps.append({"wb": wb, "ah": ah, "bpack": bp})
    res = run_bass_kernel_spmd(
        nc, in_maps, core_ids=list(range(N_CORES)), trace=trace, trace_cores=trace_cores
    )
    out = np.empty((D, D), np.float32)
    for c in range(N_CORES):
        out[ROWS_PC * c : ROWS_PC * (c + 1), :] = unpack_out(res.results[c]["outt"])
    return out, res


def kernel(W, A_blocks, B_blocks):
    out, _ = run(W, A_blocks, B_blocks, trace=False)
    return out
